# revision 21
# baseline (speedup 1.0000x reference)
"""GCN layer (gather + segment-sum + degree norm) on 8 trn2 NeuronCores.

Sharding: destination nodes across cores (12500/core). Host buckets edges
by dst range, windows of 128 dst nodes, groups window edges by src table
chunk (32768 rows, int16 gather index limit), sorts groups by src for HBM
locality, pads each (window, chunk) group to a multiple of 128 with
idx=0 / dst=-1 sentinels, uniform K per (window, chunk) across cores
(single SPMD NEFF).

Launch 1 (per core): out-degree counts for the core's node slice via
one-hot [128, K, 128] + K ones-matmuls per window (PE reduces slots and
partitions in PSUM). For most windows the srcl broadcast is materialized
on the Activation engine so the DVE is_equal runs on packed 2-byte
operands (2x perf mode). h2 = h * rsqrt(max(od,1)) cast to bf16. Host
concatenates slices into the [100000, 256] bf16 gather table (col 128 =
1.0 constant).

Launch 2 (per core): windows are processed in groups of G with
double-buffered msgs tiles; gathers are issued in subcols-column calls
(<=512 indices — a 1024-descriptor call exactly fills the SWDGE ring
carveout and serializes; >1024 crashes the ucode) round-robined over 4
SWDGE queues (single queue drain limits throughput to ~7.4ns/row; with
4 queues the ~3.5ns/row Pool-engine descriptor generation is the
bottleneck). Per window: one-hot [128, Kw, 128] via is_equal; Kw matmuls
accumulate onehot^T @ row[0:129] into PSUM [128, 129] (col 128 counts
in-degree via the table's ones column); out = agg * rsqrt(max(id,1)).
"""

import numpy as np
import ml_dtypes

import concourse.bass as bass
import concourse.bacc as bacc
import concourse.mybir as mybir
import concourse.tile as tile
from concourse.bass_utils import run_bass_kernel_spmd

N_SRC = 60000
N_DST = 40000
N_NODES = N_SRC + N_DST
D = 128
C = 8
P = 128
NPC = N_NODES // C
WN = 128
NW = (NPC + WN - 1) // WN
NPC_PAD = NW * WN
ELEM = 256               # bf16 row: h*od_r [0:128] | 1.0 | zeros
CHUNK = 32768
NCHUNK = (N_NODES + CHUNK - 1) // CHUNK
G = 5                    # windows per gather group
TRIM128 = True           # non-x128 num_idxs hits a gather-ucode slow path
                         # (~+350us/iter), outweighing the ~10% descriptor trim

f32 = mybir.dt.float32
bf16 = mybir.dt.bfloat16
i16 = mybir.dt.int16
BF = ml_dtypes.bfloat16


# ---------------------------------------------------------------- host packing
def _pack_degree(src_idx):
    order = np.argsort(src_idx, kind="stable")
    s_src = src_idx[order]
    core_of = s_src // NPC
    win_of = (s_src - core_of * NPC) // WN

    counts = np.zeros((C, NW), dtype=np.int64)
    np.add.at(counts, (core_of, win_of), 1)
    KWs = [max(int(-(-counts[:, w].max() // P)), 1) for w in range(NW)]

    cols = sum(KWs)
    srcl = np.full((C, P, cols), -1.0, dtype=np.float32)
    col0 = np.concatenate([[0], np.cumsum(KWs)])
    starts = np.concatenate([[0], np.cumsum(counts.reshape(-1))])
    for c in range(C):
        for w in range(NW):
            gi = c * NW + w
            lo, hi = starts[gi], starts[gi + 1]
            if hi == lo:
                continue
            g = s_src[lo:hi] - (c * NPC + w * WN)
            n = hi - lo
            K = KWs[w]
            pj = np.arange(n) // K
            kj = np.arange(n) % K
            srcl[c, pj, col0[w] + kj] = g
    return KWs, col0, srcl.astype(BF)


class MainPack:
    """Grouped packing for the aggregation launch.

    Attributes:
      Kwc [NW, NCHUNK]: columns per (window, chunk), uniform across cores.
      kcol0w [NW]: dstl column offset per window (window-major, chunk-minor
        inside the window — matches slot sort order).
      groups: list per group g of dict ch -> (icol0, Kc, ccol) where icol0
        is the idx-array column offset (x8 wrap), Kc the column count of
        the (g, ch) gather call, ccol its column offset in the group msgs
        tile.
      mcols [NW]: list of msgs-tile column index (within the group tile)
        for each of the window's Kw one-hot columns (chunk-major order).
      MGC: max total columns of any group's msgs tile.
      idxs [C, P, icols]: int16 gather indices (wrapped+replicated).
      dstl [C, P, kcols]: bf16 dst-local values (or -1) per slot.
    """


def _pack_main(src_idx, dst_idx):
    order = np.argsort(dst_idx, kind="stable")
    s_src = src_idx[order]
    s_dst = dst_idx[order]
    core_of = s_dst // NPC
    win_of = (s_dst - core_of * NPC) // WN
    chunk_of = s_src // CHUNK

    counts = np.zeros((C, NW, NCHUNK), dtype=np.int64)
    np.add.at(counts, (core_of, win_of, chunk_of), 1)
    Kwc = np.zeros((NW, NCHUNK), dtype=np.int64)
    for w in range(NW):
        for ch in range(NCHUNK):
            m = counts[:, w, ch].max()
            Kwc[w, ch] = -(-m // P) if m > 0 else 0

    # valid slots per (w, ch): uniform across cores (max count, rounded up
    # to x16 for the idx-wrap layout). Slots beyond vwc are never gathered —
    # the one-hot (dst=-1) zeroes them in the matmul.
    vwc = np.zeros((NW, NCHUNK), dtype=np.int64)
    for w in range(NW):
        for ch in range(NCHUNK):
            m = int(counts[:, w, ch].max())
            vwc[w, ch] = -(-m // 16) * 16 if m > 0 else 0
    if TRIM128:
        vwc = Kwc * P

    pk = MainPack()
    pk.vwc = vwc
    pk.Kwc = Kwc
    Kw_tot = Kwc.sum(axis=1)
    pk.kcol0w = np.concatenate([[0], np.cumsum(Kw_tot)])
    kcols = int(Kw_tot.sum())

    ngroups = (NW + G - 1) // G
    pk.groups = []
    pk.mcols = [None] * NW
    iacc = 0
    MGC = 0
    for g in range(ngroups):
        ws = range(g * G, min((g + 1) * G, NW))
        # blocks: per (ch, w) gather call: (w, ch, icol0, v, ccol)
        blocks = []
        ccol = 0
        for ch in range(NCHUNK):
            for w in ws:
                K = int(Kwc[w, ch])
                if K == 0:
                    continue
                v = int(vwc[w, ch])
                blocks.append((w, ch, iacc, v, ccol))
                iacc += v // 16
                ccol += K
        MGC = max(MGC, ccol)
        # per-window msgs-column mapping, chunk-major within the window
        colof = {(w, ch): cc for (w, ch, _, _, cc) in blocks}
        for w in ws:
            mc = []
            for ch in range(NCHUNK):
                if (w, ch) not in colof:
                    continue
                mc.extend(colof[(w, ch)] + k
                          for k in range(int(Kwc[w, ch])))
            pk.mcols[w] = mc
        pk.groups.append(blocks)
    pk.MGC = MGC
    pk.icols = iacc

    idxs = np.zeros((C, P, iacc), dtype=np.int16)
    dstl = np.full((C, P, kcols), -1.0, dtype=np.float32)

    starts2 = np.concatenate([[0], np.cumsum(counts.sum(axis=2).reshape(-1))])
    for c in range(C):
        slot_idx = {}
        for w in range(NW):
            gi = c * NW + w
            lo, hi = starts2[gi], starts2[gi + 1]
            g_src = s_src[lo:hi]
            g_dst = s_dst[lo:hi]
            so = np.argsort(g_src, kind="stable")
            g_src = g_src[so]
            g_dst = g_dst[so]
            g_ch = g_src // CHUNK
            kb = int(pk.kcol0w[w])
            jcol = 0
            for ch in range(NCHUNK):
                K = int(Kwc[w, ch])
                if K == 0:
                    continue
                v = int(vwc[w, ch])
                sel = g_ch == ch
                n = int(sel.sum())
                loc = np.zeros(v, dtype=np.int64)
                loc[:n] = g_src[sel] - ch * CHUNK
                dl = np.full(K * P, -1.0, dtype=np.float32)
                dl[:n] = (g_dst[sel] - (c * NPC + w * WN)).astype(np.float32)
                slot_idx[(w, ch)] = loc
                # dstl: window-major, chunk-minor columns
                dstl[c, np.arange(K * P) % P,
                     kb + jcol + np.arange(K * P) // P] = dl
                jcol += K
        for blocks in pk.groups:
            for w, ch, io, v, _ in blocks:
                seg = slot_idx[(w, ch)]
                assert seg.shape[0] == v
                wr = seg.astype(np.int16).reshape(v // 16, 16).T
                idxs[c, :, io: io + v // 16] = np.tile(wr, (8, 1))
    pk.idxs = idxs
    pk.dstl = dstl.astype(BF)
    return pk


# ---------------------------------------------------------------- bass builders
def _build_degree_nc(KWs, col0, repeat=1, act_frac=0.77):
    """Out-degree histogram via one-hot + K ones-matmuls per window (PE does
    the slot+partition reduce in PSUM). For act_frac of the windows, the
    broadcast of srcl along WN is materialized on the Activation engine so
    the DVE is_equal sees packed 2-byte operands (2x perf mode)."""
    nc = bacc.Bacc("TRN2", target_bir_lowering=False)
    cols = int(col0[-1])
    srcl_d = nc.dram_tensor("srcl", [P, cols], bf16, kind="ExternalInput")
    iota_d = nc.dram_tensor("iota", [P, WN], bf16, kind="ExternalInput")
    h_d = nc.dram_tensor("h_slice", [NPC_PAD, D], f32, kind="ExternalInput")
    h2_d = nc.dram_tensor("h2s", [NPC_PAD, D], bf16, kind="ExternalOutput")
    KWmax = max(KWs)

    with tile.TileContext(nc) as tc:
        with (
            tc.tile_pool(name="cst", bufs=1) as cst,
            tc.tile_pool(name="work", bufs=3) as wk,
            tc.tile_pool(name="psum", bufs=4, space="PSUM") as ps,
        ):
            srcl = cst.tile([P, cols], bf16)
            nc.sync.dma_start(srcl[:], srcl_d[:])
            iota = cst.tile([P, WN], bf16)
            nc.sync.dma_start(iota[:], iota_d[:])
            ones = cst.tile([P, 1], bf16)
            nc.vector.memset(ones[:], 1.0)
            iotak = cst.tile([P, KWmax, WN], bf16)
            nc.vector.tensor_copy(
                iotak[:], iota[:, None, :].to_broadcast([P, KWmax, WN])
            )

            def body(_=None):
                for w in range(NW):
                    K = KWs[w]
                    c0 = int(col0[w])
                    oh = wk.tile([P, KWmax, WN], bf16, tag="oh")
                    if (w % 13) < round(act_frac * 13):
                        rep = wk.tile([P, KWmax, WN], bf16, tag="rep")
                        nc.scalar.activation(
                            rep[:, 0:K, :],
                            srcl[:, c0 : c0 + K, None].to_broadcast([P, K, WN]),
                            mybir.ActivationFunctionType.Identity,
                        )
                        nc.vector.tensor_tensor(
                            out=oh[:, 0:K, :],
                            in0=rep[:, 0:K, :],
                            in1=iotak[:, 0:K, :],
                            op=mybir.AluOpType.is_equal,
                        )
                    else:
                        nc.vector.tensor_tensor(
                            out=oh[:, 0:K, :],
                            in0=srcl[:, c0 : c0 + K, None].to_broadcast(
                                [P, K, WN]),
                            in1=iota[:, None, :].to_broadcast([P, K, WN]),
                            op=mybir.AluOpType.is_equal,
                        )
                    od_ps = ps.tile([WN, 1], f32, space="PSUM")
                    for k in range(K):
                        nc.tensor.matmul(
                            od_ps[:], lhsT=oh[:, k, :], rhs=ones[:],
                            start=(k == 0), stop=(k == K - 1),
                        )
                    clamped = wk.tile([WN, 1], f32, tag="cl")
                    nc.vector.tensor_scalar_max(clamped[:], od_ps[:], 1.0)
                    sq = wk.tile([WN, 1], f32, tag="sq")
                    nc.scalar.activation(
                        sq[:], clamped[:], mybir.ActivationFunctionType.Sqrt
                    )
                    odr = wk.tile([WN, 1], f32, tag="odr")
                    nc.vector.reciprocal(odr[:], sq[:])
                    h_win = wk.tile([WN, D], f32, tag="hwin")
                    nc.sync.dma_start(h_win[:], h_d[w * WN : (w + 1) * WN, :])
                    h2_win = wk.tile([WN, D], bf16, tag="h2win")
                    nc.vector.tensor_scalar_mul(
                        h2_win[:], h_win[:], odr[:, 0:1]
                    )
                    nc.sync.dma_start(h2_d[w * WN : (w + 1) * WN, :], h2_win[:])

            if repeat > 1:
                with tc.For_i(0, repeat, 1):
                    body()
            else:
                body()
    nc.compile()
    return nc


def _build_main_nc(pk, repeat=1, parts="all", subcols=4, nqueues=4,
                   scratch=16384, single_packet=True):
    nc = bacc.Bacc("TRN2", target_bir_lowering=False,
                   num_swdge_queues=nqueues,
                   dynamic_dma_scratch_size=scratch)
    kcols = int(pk.kcol0w[-1])
    icols = int(pk.icols)
    KWmax = int(pk.Kwc.sum(axis=1).max())
    h2_d = nc.dram_tensor("h2", [N_NODES, ELEM], bf16, kind="ExternalInput")
    idx_d = nc.dram_tensor("idxs", [P, icols], i16, kind="ExternalInput")
    dstl_d = nc.dram_tensor("dstl", [P, kcols], bf16, kind="ExternalInput")
    iota_d = nc.dram_tensor("iota", [P, WN], bf16, kind="ExternalInput")
    out_d = nc.dram_tensor("out_slice", [NPC_PAD, D], f32, kind="ExternalOutput")

    chunk_rows = [min(CHUNK, N_NODES - ch * CHUNK) for ch in range(NCHUNK)]

    with tile.TileContext(nc) as tc:
        with (
            tc.tile_pool(name="cst", bufs=1) as cst,
            tc.tile_pool(name="msgs", bufs=2) as mp,
            tc.tile_pool(name="work", bufs=3) as wk,
            tc.tile_pool(name="psum", bufs=4, space="PSUM") as ps,
        ):
            idxs = cst.tile([P, icols], i16)
            nc.sync.dma_start(idxs[:], idx_d[:])
            dstl = cst.tile([P, kcols], bf16)
            nc.sync.dma_start(dstl[:], dstl_d[:])
            iota = cst.tile([P, WN], bf16)
            nc.sync.dma_start(iota[:], iota_d[:])

            qctr = [0]
            # trailing slots of trimmed gather calls stay unwritten; clear the
            # two ring buffers once so the matmul never reads NaN garbage
            for _ in range(2):
                mz = mp.tile([P, pk.MGC, ELEM], bf16, tag="msgs")
                nc.vector.memset(mz[:], 0.0)

            def body(_=None):
                for g, blocks in enumerate(pk.groups):
                    ws = range(g * G, min((g + 1) * G, NW))
                    msgs = mp.tile([P, pk.MGC, ELEM], bf16, tag="msgs")
                    for w, ch, io, v, ccol in blocks:
                        K = int(pk.Kwc[w, ch])
                        nc.gpsimd.dma_gather(
                            out_ap=msgs[:, ccol : ccol + K, :],
                            in_ap=h2_d[
                                ch * CHUNK : ch * CHUNK + chunk_rows[ch], :
                            ],
                            idxs_ap=idxs[:, io : io + v // 16],
                            num_idxs=v,
                            num_idxs_reg=v,
                            elem_size=ELEM,
                            queue_num=qctr[0] % nqueues,
                            single_packet=single_packet,
                        )
                        qctr[0] += 1
                    if parts == "gather":
                        continue
                    for w in ws:
                        Kw = int(pk.Kwc[w].sum())
                        kb = int(pk.kcol0w[w])
                        oh = wk.tile([P, KWmax, WN], bf16, tag="oh")
                        nc.vector.tensor_tensor(
                            out=oh[:, 0:Kw, :],
                            in0=dstl[:, kb : kb + Kw, None].to_broadcast(
                                [P, Kw, WN]
                            ),
                            in1=iota[:, None, :].to_broadcast([P, Kw, WN]),
                            op=mybir.AluOpType.is_equal,
                        )
                        if parts == "onehot":
                            continue
                        acc = ps.tile([WN, D + 1], f32, space="PSUM")
                        for j, mc in enumerate(pk.mcols[w]):
                            nc.tensor.matmul(
                                acc[:],
                                lhsT=oh[:, j, :],
                                rhs=msgs[:, mc, 0 : D + 1],
                                start=(j == 0),
                                stop=(j == Kw - 1),
                            )
                        clamped = wk.tile([WN, 1], f32, tag="cl")
                        nc.vector.tensor_scalar_max(
                            clamped[:], acc[:, D : D + 1], 1.0
                        )
                        sq = wk.tile([WN, 1], f32, tag="sq")
                        nc.scalar.activation(
                            sq[:], clamped[:],
                            mybir.ActivationFunctionType.Sqrt
                        )
                        rsq = wk.tile([WN, 1], f32, tag="rsq")
                        nc.vector.reciprocal(rsq[:], sq[:])
                        fin = wk.tile([WN, D], f32, tag="fin")
                        nc.vector.tensor_scalar_mul(
                            fin[:], acc[:, 0:D], rsq[:, 0:1]
                        )
                        nc.sync.dma_start(
                            out_d[w * WN : (w + 1) * WN, :], fin[:]
                        )

            if repeat > 1:
                with tc.For_i(0, repeat, 1):
                    body()
            else:
                body()
    nc.compile()
    return nc


# ---------------------------------------------------------------- entry point
def kernel(src_embedding, dst_embedding, src_idx, dst_idx):
    src_embedding = np.asarray(src_embedding, dtype=np.float32)
    dst_embedding = np.asarray(dst_embedding, dtype=np.float32)
    src_idx = np.asarray(src_idx).astype(np.int64)
    dst_idx = np.asarray(dst_idx).astype(np.int64)

    iota_np = np.broadcast_to(
        np.arange(WN, dtype=np.float32), (P, WN)
    ).astype(BF)
    h_full = np.concatenate([src_embedding, dst_embedding], axis=0)

    # ---- launch 1: out-degree rsqrt + table scale on device
    dKWs, dcol0, srcl = _pack_degree(src_idx)
    nc1 = _build_degree_nc(dKWs, dcol0)
    in_maps1 = []
    for c in range(C):
        hs = np.zeros((NPC_PAD, D), dtype=np.float32)
        hs[:NPC] = h_full[c * NPC : (c + 1) * NPC]
        in_maps1.append(
            {"srcl": np.ascontiguousarray(srcl[c]), "iota": iota_np,
             "h_slice": hs}
        )
    res1 = run_bass_kernel_spmd(nc1, in_maps1, core_ids=list(range(C)))
    kernel.last_res1 = res1

    # ---- host glue: assemble bf16 gather table (layout only)
    h2 = np.zeros((N_NODES, ELEM), dtype=BF)
    for c in range(C):
        h2[c * NPC : (c + 1) * NPC, :D] = res1.results[c]["h2s"][:NPC]
    h2[:, D] = np.float32(1.0)

    # ---- launch 2: gather + aggregate + normalize
    pk = _pack_main(src_idx, dst_idx)
    nc2 = _build_main_nc(pk)
    in_maps2 = [
        {
            "h2": h2,
            "idxs": np.ascontiguousarray(pk.idxs[c]),
            "dstl": np.ascontiguousarray(pk.dstl[c]),
            "iota": iota_np,
        }
        for c in range(C)
    ]
    res2 = run_bass_kernel_spmd(nc2, in_maps2, core_ids=list(range(C)))
    kernel.last_res2 = res2
    out = np.concatenate(
        [res2.results[c]["out_slice"][:NPC] for c in range(C)], axis=0
    )
    return out


# revision 22
# speedup vs baseline: 1.5029x; 1.5029x over previous
"""GCN layer (gather + segment-sum + degree norm) on 8 trn2 NeuronCores.

Sharding: destination nodes across cores (12500/core). Host buckets edges
by dst range, windows of 128 dst nodes, groups window edges by src table
chunk (32768 rows, int16 gather index limit), sorts groups by src for HBM
locality, pads each (window, chunk) group to a multiple of 128 with
idx=0 / dst=-1 sentinels, uniform K per (window, chunk) across cores
(single SPMD NEFF).

Launch 1 (per core): out-degree counts for the core's node slice via
one-hot [128, K, 128] + K ones-matmuls per window (PE reduces slots and
partitions in PSUM). For most windows the srcl broadcast is materialized
on the Activation engine so the DVE is_equal runs on packed 2-byte
operands (2x perf mode). h2 = h * rsqrt(max(od,1)) cast to bf16. Host
concatenates slices into the [100000, 256] bf16 gather table (col 128 =
1.0 constant).

Launch 2 (per core): windows are processed in groups of G with
double-buffered msgs tiles; gathers are issued in subcols-column calls
(<=512 indices — a 1024-descriptor call exactly fills the SWDGE ring
carveout and serializes; >1024 crashes the ucode) round-robined over 4
SWDGE queues (single queue drain limits throughput to ~7.4ns/row; with
4 queues the ~3.5ns/row Pool-engine descriptor generation is the
bottleneck). Per window: one-hot [128, Kw, 128] via is_equal; Kw matmuls
accumulate onehot^T @ row[0:129] into PSUM [128, 129] (col 128 counts
in-degree via the table's ones column); out = agg * rsqrt(max(id,1)).
"""

import numpy as np
import ml_dtypes

import concourse.bass as bass
import concourse.bacc as bacc
import concourse.mybir as mybir
import concourse.tile as tile
from concourse.bass_utils import run_bass_kernel_spmd

N_SRC = 60000
N_DST = 40000
N_NODES = N_SRC + N_DST
D = 128
C = 8
P = 128
NPC = N_NODES // C
WN = 128
NW = (NPC + WN - 1) // WN
NPC_PAD = NW * WN
ELEM = 256               # bf16 row: h*od_r [0:128] | 1.0 | zeros
CHUNK = 32768
NCHUNK = (N_NODES + CHUNK - 1) // CHUNK
G = 5                    # windows per gather group
TRIM128 = True           # non-x128 num_idxs hits a gather-ucode slow path
                         # (~+350us/iter), outweighing the ~10% descriptor trim

f32 = mybir.dt.float32
bf16 = mybir.dt.bfloat16
i16 = mybir.dt.int16
BF = ml_dtypes.bfloat16


# ---------------------------------------------------------------- host packing
def _pack_degree(src_idx):
    order = np.argsort(src_idx, kind="stable")
    s_src = src_idx[order]
    core_of = s_src // NPC
    win_of = (s_src - core_of * NPC) // WN

    counts = np.zeros((C, NW), dtype=np.int64)
    np.add.at(counts, (core_of, win_of), 1)
    KWs = [max(int(-(-counts[:, w].max() // P)), 1) for w in range(NW)]

    cols = sum(KWs)
    srcl = np.full((C, P, cols), -1.0, dtype=np.float32)
    col0 = np.concatenate([[0], np.cumsum(KWs)])
    starts = np.concatenate([[0], np.cumsum(counts.reshape(-1))])
    for c in range(C):
        for w in range(NW):
            gi = c * NW + w
            lo, hi = starts[gi], starts[gi + 1]
            if hi == lo:
                continue
            g = s_src[lo:hi] - (c * NPC + w * WN)
            n = hi - lo
            K = KWs[w]
            pj = np.arange(n) // K
            kj = np.arange(n) % K
            srcl[c, pj, col0[w] + kj] = g
    return KWs, col0, srcl.astype(BF)


class MainPack:
    """Grouped packing for the aggregation launch.

    Attributes:
      Kwc [NW, NCHUNK]: columns per (window, chunk), uniform across cores.
      kcol0w [NW]: dstl column offset per window (window-major, chunk-minor
        inside the window — matches slot sort order).
      groups: list per group g of dict ch -> (icol0, Kc, ccol) where icol0
        is the idx-array column offset (x8 wrap), Kc the column count of
        the (g, ch) gather call, ccol its column offset in the group msgs
        tile.
      mcols [NW]: list of msgs-tile column index (within the group tile)
        for each of the window's Kw one-hot columns (chunk-major order).
      MGC: max total columns of any group's msgs tile.
      idxs [C, P, icols]: int16 gather indices (wrapped+replicated).
      dstl [C, P, kcols]: bf16 dst-local values (or -1) per slot.
    """


def _pack_main(src_idx, dst_idx):
    order = np.argsort(dst_idx, kind="stable")
    s_src = src_idx[order]
    s_dst = dst_idx[order]
    core_of = s_dst // NPC
    win_of = (s_dst - core_of * NPC) // WN
    chunk_of = s_src // CHUNK

    counts = np.zeros((C, NW, NCHUNK), dtype=np.int64)
    np.add.at(counts, (core_of, win_of, chunk_of), 1)
    Kwc = np.zeros((NW, NCHUNK), dtype=np.int64)
    for w in range(NW):
        for ch in range(NCHUNK):
            m = counts[:, w, ch].max()
            Kwc[w, ch] = -(-m // P) if m > 0 else 0

    # valid slots per (w, ch): uniform across cores (max count, rounded up
    # to x16 for the idx-wrap layout). Slots beyond vwc are never gathered —
    # the one-hot (dst=-1) zeroes them in the matmul.
    vwc = np.zeros((NW, NCHUNK), dtype=np.int64)
    for w in range(NW):
        for ch in range(NCHUNK):
            m = int(counts[:, w, ch].max())
            vwc[w, ch] = -(-m // 16) * 16 if m > 0 else 0
    if TRIM128:
        vwc = Kwc * P

    pk = MainPack()
    pk.vwc = vwc
    pk.Kwc = Kwc
    Kw_tot = Kwc.sum(axis=1)
    pk.kcol0w = np.concatenate([[0], np.cumsum(Kw_tot)])
    kcols = int(Kw_tot.sum())

    ngroups = (NW + G - 1) // G
    pk.groups = []
    pk.mcols = [None] * NW
    iacc = 0
    MGC = 0
    for g in range(ngroups):
        ws = range(g * G, min((g + 1) * G, NW))
        # blocks: per (ch, w) gather call: (w, ch, icol0, v, ccol)
        blocks = []
        ccol = 0
        for ch in range(NCHUNK):
            for w in ws:
                K = int(Kwc[w, ch])
                if K == 0:
                    continue
                v = int(vwc[w, ch])
                blocks.append((w, ch, iacc, v, ccol))
                iacc += v // 16
                ccol += K
        MGC = max(MGC, ccol)
        # per-window msgs-column mapping, chunk-major within the window
        colof = {(w, ch): cc for (w, ch, _, _, cc) in blocks}
        for w in ws:
            mc = []
            for ch in range(NCHUNK):
                if (w, ch) not in colof:
                    continue
                mc.extend(colof[(w, ch)] + k
                          for k in range(int(Kwc[w, ch])))
            pk.mcols[w] = mc
        pk.groups.append(blocks)
    pk.MGC = MGC
    pk.icols = iacc

    idxs = np.zeros((C, P, iacc), dtype=np.int16)
    dstl = np.full((C, P, kcols), -1.0, dtype=np.float32)

    starts2 = np.concatenate([[0], np.cumsum(counts.sum(axis=2).reshape(-1))])
    for c in range(C):
        slot_idx = {}
        for w in range(NW):
            gi = c * NW + w
            lo, hi = starts2[gi], starts2[gi + 1]
            g_src = s_src[lo:hi]
            g_dst = s_dst[lo:hi]
            so = np.argsort(g_src, kind="stable")
            g_src = g_src[so]
            g_dst = g_dst[so]
            g_ch = g_src // CHUNK
            kb = int(pk.kcol0w[w])
            jcol = 0
            for ch in range(NCHUNK):
                K = int(Kwc[w, ch])
                if K == 0:
                    continue
                v = int(vwc[w, ch])
                sel = g_ch == ch
                n = int(sel.sum())
                loc = np.zeros(v, dtype=np.int64)
                loc[:n] = g_src[sel] - ch * CHUNK
                dl = np.full(K * P, -1.0, dtype=np.float32)
                dl[:n] = (g_dst[sel] - (c * NPC + w * WN)).astype(np.float32)
                slot_idx[(w, ch)] = loc
                # dstl: window-major, chunk-minor columns
                dstl[c, np.arange(K * P) % P,
                     kb + jcol + np.arange(K * P) // P] = dl
                jcol += K
        for blocks in pk.groups:
            for w, ch, io, v, _ in blocks:
                seg = slot_idx[(w, ch)]
                assert seg.shape[0] == v
                wr = seg.astype(np.int16).reshape(v // 16, 16).T
                idxs[c, :, io: io + v // 16] = np.tile(wr, (8, 1))
    pk.idxs = idxs
    pk.dstl = dstl.astype(BF)
    return pk


# ---------------------------------------------------------------- bass builders
def _build_degree_nc(KWs, col0, repeat=1, act_frac=0.77):
    """Out-degree histogram via one-hot + K ones-matmuls per window (PE does
    the slot+partition reduce in PSUM). For act_frac of the windows, the
    broadcast of srcl along WN is materialized on the Activation engine so
    the DVE is_equal sees packed 2-byte operands (2x perf mode)."""
    nc = bacc.Bacc("TRN2", target_bir_lowering=False)
    cols = int(col0[-1])
    srcl_d = nc.dram_tensor("srcl", [P, cols], bf16, kind="ExternalInput")
    iota_d = nc.dram_tensor("iota", [P, WN], bf16, kind="ExternalInput")
    h_d = nc.dram_tensor("h_slice", [NPC_PAD, D], f32, kind="ExternalInput")
    h2_d = nc.dram_tensor("h2s", [NPC_PAD, D], bf16, kind="ExternalOutput")
    KWmax = max(KWs)

    with tile.TileContext(nc) as tc:
        with (
            tc.tile_pool(name="cst", bufs=1) as cst,
            tc.tile_pool(name="work", bufs=3) as wk,
            tc.tile_pool(name="psum", bufs=4, space="PSUM") as ps,
        ):
            srcl = cst.tile([P, cols], bf16)
            nc.sync.dma_start(srcl[:], srcl_d[:])
            iota = cst.tile([P, WN], bf16)
            nc.sync.dma_start(iota[:], iota_d[:])
            ones = cst.tile([P, 1], bf16)
            nc.vector.memset(ones[:], 1.0)
            iotak = cst.tile([P, KWmax, WN], bf16)
            nc.vector.tensor_copy(
                iotak[:], iota[:, None, :].to_broadcast([P, KWmax, WN])
            )

            def body(_=None):
                for w in range(NW):
                    K = KWs[w]
                    c0 = int(col0[w])
                    oh = wk.tile([P, KWmax, WN], bf16, tag="oh")
                    if (w % 13) < round(act_frac * 13):
                        rep = wk.tile([P, KWmax, WN], bf16, tag="rep")
                        nc.scalar.activation(
                            rep[:, 0:K, :],
                            srcl[:, c0 : c0 + K, None].to_broadcast([P, K, WN]),
                            mybir.ActivationFunctionType.Identity,
                        )
                        nc.vector.tensor_tensor(
                            out=oh[:, 0:K, :],
                            in0=rep[:, 0:K, :],
                            in1=iotak[:, 0:K, :],
                            op=mybir.AluOpType.is_equal,
                        )
                    else:
                        nc.vector.tensor_tensor(
                            out=oh[:, 0:K, :],
                            in0=srcl[:, c0 : c0 + K, None].to_broadcast(
                                [P, K, WN]),
                            in1=iota[:, None, :].to_broadcast([P, K, WN]),
                            op=mybir.AluOpType.is_equal,
                        )
                    od_ps = ps.tile([WN, 1], f32, space="PSUM")
                    for k in range(K):
                        nc.tensor.matmul(
                            od_ps[:], lhsT=oh[:, k, :], rhs=ones[:],
                            start=(k == 0), stop=(k == K - 1),
                        )
                    clamped = wk.tile([WN, 1], f32, tag="cl")
                    nc.vector.tensor_scalar_max(clamped[:], od_ps[:], 1.0)
                    sq = wk.tile([WN, 1], f32, tag="sq")
                    nc.scalar.activation(
                        sq[:], clamped[:], mybir.ActivationFunctionType.Sqrt
                    )
                    odr = wk.tile([WN, 1], f32, tag="odr")
                    nc.vector.reciprocal(odr[:], sq[:])
                    h_win = wk.tile([WN, D], f32, tag="hwin")
                    nc.sync.dma_start(h_win[:], h_d[w * WN : (w + 1) * WN, :])
                    h2_win = wk.tile([WN, D], bf16, tag="h2win")
                    nc.vector.tensor_scalar_mul(
                        h2_win[:], h_win[:], odr[:, 0:1]
                    )
                    nc.sync.dma_start(h2_d[w * WN : (w + 1) * WN, :], h2_win[:])

            if repeat > 1:
                with tc.For_i(0, repeat, 1):
                    body()
            else:
                body()
    nc.compile()
    return nc


def _build_main_nc(pk, repeat=1, parts="all", subcols=4, nqueues=4,
                   scratch=16384, single_packet=True):
    nc = bacc.Bacc("TRN2", target_bir_lowering=False,
                   num_swdge_queues=nqueues,
                   dynamic_dma_scratch_size=scratch)
    kcols = int(pk.kcol0w[-1])
    icols = int(pk.icols)
    KWmax = int(pk.Kwc.sum(axis=1).max())
    h2_d = nc.dram_tensor("h2", [N_NODES, ELEM], bf16, kind="ExternalInput")
    idx_d = nc.dram_tensor("idxs", [P, icols], i16, kind="ExternalInput")
    dstl_d = nc.dram_tensor("dstl", [P, kcols], bf16, kind="ExternalInput")
    iota_d = nc.dram_tensor("iota", [P, WN], bf16, kind="ExternalInput")
    out_d = nc.dram_tensor("out_slice", [NPC_PAD, D], f32, kind="ExternalOutput")

    chunk_rows = [min(CHUNK, N_NODES - ch * CHUNK) for ch in range(NCHUNK)]

    with tile.TileContext(nc) as tc:
        with (
            tc.tile_pool(name="cst", bufs=1) as cst,
            tc.tile_pool(name="msgs", bufs=2) as mp,
            tc.tile_pool(name="work", bufs=3) as wk,
            tc.tile_pool(name="psum", bufs=4, space="PSUM") as ps,
        ):
            idxs = cst.tile([P, icols], i16)
            nc.sync.dma_start(idxs[:], idx_d[:])
            dstl = cst.tile([P, kcols], bf16)
            nc.sync.dma_start(dstl[:], dstl_d[:])
            iota = cst.tile([P, WN], bf16)
            nc.sync.dma_start(iota[:], iota_d[:])

            qctr = [0]
            if not TRIM128:
                # trailing slots of trimmed gather calls stay unwritten; clear
                # the ring buffers once so the matmul never reads NaN garbage
                for _ in range(2):
                    mz = mp.tile([P, pk.MGC, ELEM], bf16, tag="msgs")
                    nc.vector.memset(mz[:], 0.0)

            def body(_=None):
                for g, blocks in enumerate(pk.groups):
                    ws = range(g * G, min((g + 1) * G, NW))
                    msgs = mp.tile([P, pk.MGC, ELEM], bf16, tag="msgs")
                    for w, ch, io, v, ccol in blocks:
                        K = int(pk.Kwc[w, ch])
                        nc.gpsimd.dma_gather(
                            out_ap=msgs[:, ccol : ccol + K, :],
                            in_ap=h2_d[
                                ch * CHUNK : ch * CHUNK + chunk_rows[ch], :
                            ],
                            idxs_ap=idxs[:, io : io + v // 16],
                            num_idxs=v,
                            num_idxs_reg=v,
                            elem_size=ELEM,
                            queue_num=qctr[0] % nqueues,
                            single_packet=single_packet,
                        )
                        qctr[0] += 1
                    if parts == "gather":
                        continue
                    for w in ws:
                        Kw = int(pk.Kwc[w].sum())
                        kb = int(pk.kcol0w[w])
                        oh = wk.tile([P, KWmax, WN], bf16, tag="oh")
                        nc.vector.tensor_tensor(
                            out=oh[:, 0:Kw, :],
                            in0=dstl[:, kb : kb + Kw, None].to_broadcast(
                                [P, Kw, WN]
                            ),
                            in1=iota[:, None, :].to_broadcast([P, Kw, WN]),
                            op=mybir.AluOpType.is_equal,
                        )
                        if parts == "onehot":
                            continue
                        acc = ps.tile([WN, D + 1], f32, space="PSUM")
                        for j, mc in enumerate(pk.mcols[w]):
                            nc.tensor.matmul(
                                acc[:],
                                lhsT=oh[:, j, :],
                                rhs=msgs[:, mc, 0 : D + 1],
                                start=(j == 0),
                                stop=(j == Kw - 1),
                            )
                        clamped = wk.tile([WN, 1], f32, tag="cl")
                        nc.vector.tensor_scalar_max(
                            clamped[:], acc[:, D : D + 1], 1.0
                        )
                        sq = wk.tile([WN, 1], f32, tag="sq")
                        nc.scalar.activation(
                            sq[:], clamped[:],
                            mybir.ActivationFunctionType.Sqrt
                        )
                        rsq = wk.tile([WN, 1], f32, tag="rsq")
                        nc.vector.reciprocal(rsq[:], sq[:])
                        fin = wk.tile([WN, D], f32, tag="fin")
                        nc.vector.tensor_scalar_mul(
                            fin[:], acc[:, 0:D], rsq[:, 0:1]
                        )
                        nc.sync.dma_start(
                            out_d[w * WN : (w + 1) * WN, :], fin[:]
                        )

            if repeat > 1:
                with tc.For_i(0, repeat, 1):
                    body()
            else:
                body()
    nc.compile()
    return nc


# ---------------------------------------------------------------- entry point
def kernel(src_embedding, dst_embedding, src_idx, dst_idx):
    src_embedding = np.asarray(src_embedding, dtype=np.float32)
    dst_embedding = np.asarray(dst_embedding, dtype=np.float32)
    src_idx = np.asarray(src_idx).astype(np.int64)
    dst_idx = np.asarray(dst_idx).astype(np.int64)

    iota_np = np.broadcast_to(
        np.arange(WN, dtype=np.float32), (P, WN)
    ).astype(BF)
    h_full = np.concatenate([src_embedding, dst_embedding], axis=0)

    # ---- launch 1: out-degree rsqrt + table scale on device
    dKWs, dcol0, srcl = _pack_degree(src_idx)
    nc1 = _build_degree_nc(dKWs, dcol0)
    in_maps1 = []
    for c in range(C):
        hs = np.zeros((NPC_PAD, D), dtype=np.float32)
        hs[:NPC] = h_full[c * NPC : (c + 1) * NPC]
        in_maps1.append(
            {"srcl": np.ascontiguousarray(srcl[c]), "iota": iota_np,
             "h_slice": hs}
        )
    res1 = run_bass_kernel_spmd(nc1, in_maps1, core_ids=list(range(C)))
    kernel.last_res1 = res1

    # ---- host glue: assemble bf16 gather table (layout only)
    h2 = np.zeros((N_NODES, ELEM), dtype=BF)
    for c in range(C):
        h2[c * NPC : (c + 1) * NPC, :D] = res1.results[c]["h2s"][:NPC]
    h2[:, D] = np.float32(1.0)

    # ---- launch 2: gather + aggregate + normalize
    pk = _pack_main(src_idx, dst_idx)
    nc2 = _build_main_nc(pk)
    in_maps2 = [
        {
            "h2": h2,
            "idxs": np.ascontiguousarray(pk.idxs[c]),
            "dstl": np.ascontiguousarray(pk.dstl[c]),
            "iota": iota_np,
        }
        for c in range(C)
    ]
    res2 = run_bass_kernel_spmd(nc2, in_maps2, core_ids=list(range(C)))
    kernel.last_res2 = res2
    out = np.concatenate(
        [res2.results[c]["out_slice"][:NPC] for c in range(C)], axis=0
    )
    return out
